# revision 7
# baseline (speedup 1.0000x reference)
"""DCBasicBlock kernel for Trainium2 (8 NeuronCores, data-parallel over batch).

Reference computation (all integer-valued f32 data):
    x [32,256,56,56], w1,w2 [256,256,3,3]
    y  = conv3x3_valid(pad_w_to60(x), w1)        # [32,256,54,58]
    y  = y[:, :, 1:53, :]                        # crop H
    z  = conv3x3_valid(pad_w_to60(y), w2)        # [32,256,50,58]
    z  = z[:, :, 1:49, :]                        # crop H
    out = relu(z[:, :, :, 1:57] + x[:, :, 4:52, :])   # [32,256,48,56]

Kernel strategy:
  - Data-parallel: 4 images per core, 8 cores; weights replicated.
  - Conv = 9 shifted matmuls over a width-60 zero-padded row-major grid;
    contraction over ci, accumulated in PSUM. Only the rows/cols that the
    final output needs are computed (conv1 rows 2..51; conv2 rows 1..48,
    cols 1..56).
  - conv1 in fp8e4m3 with DoubleRow (x in [0,7] and w1 in [0,6] are
    fp8-exact; products accumulate in fp32 PSUM, so conv1 is exact): one
    128x(2x128) matmul per kernel offset contracts all 256 channels.
  - conv1 output y holds integers up to 96768 (not fp16-exact), so split
    y = 2*yh + r with yh = rne_fp16(y*0.5) (<=48384, in fp16 range),
    r = y - 2*yh in [-33,33] (fp16-exact). Then
    conv2(y, w2) = conv2(yh, 2*w2) + conv2(r, w2), with 2*w2 pre-scaled on
    host (<=12, fp16-exact); both passes accumulate into one PSUM bank.
"""

import numpy as np

N_CORES = 8
IMGS = 4          # images per core
GW = 60           # padded grid width
C1_ROWS = 50      # conv1 grid rows (conv1 out rows 2..51)
C2_ROWS = 48      # conv2 grid rows (= final out rows)
YLEN = C1_ROWS * GW          # 3000
YBUF = YLEN + 64             # slack for shifted reads (stays zero)
XLEN = 56 * GW               # 3360

_CACHE = {}


def _build_program(seq=None):
    """Build the Bass program. `seq` is the list of image indices to process
    (default [0,1,2,3]); repeats allowed — used for slope-based timing."""
    import concourse.tile as tile
    from concourse import bacc, mybir

    if seq is None:
        seq = list(range(IMGS))

    f8 = mybir.dt.float8e4
    f16 = mybir.dt.float16
    f32 = mybir.dt.float32
    AF = mybir.ActivationFunctionType
    ALU = mybir.AluOpType
    DR = mybir.MatmulPerfMode.DoubleRow

    nc = bacc.Bacc("TRN2", target_bir_lowering=False, debug=False)
    # x8[img] free-dim layout: [slot(2: ci tile)][pos(3360)]
    x8 = nc.dram_tensor("x8", [IMGS, 128, 2 * XLEN], f8, kind="ExternalInput").ap()
    # w1q free-dim layout: [off(9)][slot(2: ci tile)][co(256)]
    w1q = nc.dram_tensor("w1q", [128, 9 * 2 * 256], f8, kind="ExternalInput").ap()
    # w2c[ci_t] free-dim layout: [set(2: 2*w2, w2)][off(9)][co(256)]
    w2c = nc.dram_tensor("w2c", [2, 128, 2 * 9 * 256], f16, kind="ExternalInput").ap()
    out = nc.dram_tensor("out", [IMGS, 256, C2_ROWS, 56], f32, kind="ExternalOutput").ap()

    # conv1 spatial tiles: (t0, nrows) over 50 rows; conv2: over 48 rows
    c1_tiles = [(t, min(8, C1_ROWS - t)) for t in range(0, C1_ROWS, 8)]
    c2_tiles = [(q, 8) for q in range(0, C2_ROWS, 8)]

    with tile.TileContext(nc) as tc:
        with (
            tc.tile_pool(name="w", bufs=1) as wpool,
            tc.tile_pool(name="x", bufs=2) as xpool,
            tc.tile_pool(name="y", bufs=1) as ypool,
            tc.tile_pool(name="o", bufs=4) as opool,
            tc.tile_pool(name="ps1", bufs=4, space="PSUM") as ps1,
            tc.tile_pool(name="ps2", bufs=4, space="PSUM") as ps2,
        ):
            w1_sb = wpool.tile([128, 9 * 2 * 256], f8, tag="w1")
            nc.sync.dma_start(w1_sb[:], w1q[:])
            w2_sb = []
            for c in range(2):
                t = wpool.tile([128, 2 * 9 * 256], f16, tag=f"w2{c}")
                nc.sync.dma_start(t[:], w2c[c])
                w2_sb.append(t)

            def w1ap(off, co_t):
                # [128, slot(2), co(128)] fp8, slot step 256 (%16 bytes ok)
                base = off * 512
                v = w1_sb[:, base:base + 512].rearrange("p (s c) -> p s c", s=2)
                return v[:, :, co_t * 128:(co_t + 1) * 128]

            def w2ap(c, s, off, co_t):
                base = s * 2304 + off * 256 + co_t * 128
                return w2_sb[c][:, base:base + 128]

            # persistent conv1-output buffers, 2 parities x {yh, r} x 2 ci tiles;
            # zeroed once — all later writes stay inside cols 0..57 of each row,
            # so pad col 58 (read by conv2 as AlignW zero-padding) stays zero.
            ybuf = {}
            for par in range(2):
                for kind in ("h", "r"):
                    for c in range(2):
                        t = ypool.tile([128, YBUF], f16, tag=f"y{kind}{par}{c}")
                        # only pad cols 58,59 of each row + the tail slack need
                        # zeros; cols 0..57 are overwritten every image
                        pad_v = t[:, :YLEN].rearrange(
                            "p (r c2) -> p r c2", c2=GW)[:, :, 58:60]
                        nc.gpsimd.memset(pad_v, 0.0)
                        nc.gpsimd.memset(t[:, YLEN:], 0.0)
                        ybuf[(par, kind, c)] = t

            for it_i, img in enumerate(seq):
                par = it_i % 2
                xt = xpool.tile([128, 2 * XLEN], f8, tag="x")
                nc.sync.dma_start(xt[:], x8[img])
                xv = xt[:].rearrange("p (s n) -> p s n", s=2)

                # ---------------- conv1 (fp8 DoubleRow) ----------------
                for co_t in range(2):
                    for (t0, nrows) in c1_tiles:
                        n = nrows * GW
                        ps = ps1.tile([128, 480], f32, tag="ps1")
                        for mm, (kh, kw) in enumerate(
                            (kh, kw) for kh in range(3) for kw in range(3)
                        ):
                            off = (2 + kh + t0) * GW + kw
                            nc.tensor.matmul(
                                ps[:, :n],
                                w1ap(kh * 3 + kw, co_t),
                                xv[:, :, off:off + n],
                                start=(mm == 0),
                                stop=(mm == 8),
                                perf_mode=DR,
                            )
                        # epilogue: yh = rne16(y*0.5); r = y - 2*yh (cols 0..57)
                        ps_v = ps[:, :n].rearrange("p (r c) -> p r c", c=GW)[:, :, 0:58]
                        sl = slice(t0 * GW, t0 * GW + n)
                        yh_v = ybuf[(par, "h", co_t)][:, sl].rearrange(
                            "p (r c) -> p r c", c=GW)[:, :, 0:58]
                        r_v = ybuf[(par, "r", co_t)][:, sl].rearrange(
                            "p (r c) -> p r c", c=GW)[:, :, 0:58]
                        nc.scalar.activation(yh_v, ps_v, AF.Copy, scale=0.5)
                        nc.vector.scalar_tensor_tensor(
                            r_v, yh_v, -2.0, ps_v, op0=ALU.mult, op1=ALU.add
                        )

                # ---------------- conv2 + residual + relu ----------------
                # z cols 1..56 only (the ones the final output uses)
                for co_t in range(2):
                    for (q0, nrows) in c2_tiles:
                        n = nrows * 56
                        ps = ps2.tile([128, 448], f32, tag="ps2")
                        mm = 0
                        for kh in range(3):
                            for kw in range(3):
                                for c in range(2):
                                    off = (kh + q0) * GW + kw + 1
                                    rh = ybuf[(par, "h", c)][:, off:off + nrows * GW]
                                    rh = rh.rearrange(
                                        "p (r c2) -> p r c2", c2=GW)[:, :, 0:56]
                                    rr = ybuf[(par, "r", c)][:, off:off + nrows * GW]
                                    rr = rr.rearrange(
                                        "p (r c2) -> p r c2", c2=GW)[:, :, 0:56]
                                    nc.tensor.matmul(
                                        ps[:, :n], w2ap(c, 0, kh * 3 + kw, co_t),
                                        rh, start=(mm == 0), stop=False,
                                    )
                                    mm += 1
                                    nc.tensor.matmul(
                                        ps[:, :n], w2ap(c, 1, kh * 3 + kw, co_t),
                                        rr, start=False, stop=(mm == 35),
                                    )
                                    mm += 1
                        # out[i, j] = relu(z[i, j] + x[i+4, j]) (z already col-shifted)
                        z_v = ps[:, :n].rearrange("p (r c) -> p r c", c=56)
                        x_v = xv[:, co_t, (q0 + 4) * GW:(q0 + 4 + nrows) * GW]
                        x_v = x_v.rearrange("p (r c) -> p r c", c=GW)[:, :, 0:56]
                        s = opool.tile([128, n], f32, tag="s")
                        s_v = s[:].rearrange("p (r c) -> p r c", c=56)
                        nc.vector.scalar_tensor_tensor(
                            s_v, z_v, 1.0, x_v, op0=ALU.mult, op1=ALU.add
                        )
                        o = opool.tile([128, n], f32, tag="o")
                        nc.scalar.activation(o[:], s[:], AF.Relu)
                        nc.sync.dma_start(
                            out[img, co_t * 128:(co_t + 1) * 128, q0:q0 + nrows, :],
                            o[:],
                        )

    nc.compile()
    return nc


def _get_program(seq=None):
    key = tuple(seq) if seq is not None else tuple(range(IMGS))
    if key not in _CACHE:
        _CACHE[key] = _build_program(list(key))
    return _CACHE[key]


def _prep_inputs(x, w1, w2):
    """Host-side layout prep (pure numpy, exact casts)."""
    import ml_dtypes

    f8 = ml_dtypes.float8_e4m3
    B = x.shape[0]
    xpad = np.zeros((B, 256, 56, GW), np.float32)
    xpad[..., :56] = x
    # [B, 256, 3360] -> [B, 2, 128, 3360] -> [B, 128, 2, 3360]
    x8 = (
        xpad.reshape(B, 2, 128, XLEN)
        .transpose(0, 2, 1, 3)
        .reshape(B, 128, 2 * XLEN)
        .astype(f8)
    )

    # w1 [co, ci, kh, kw] -> [ki(128), off(9), slot(2), co(256)]
    w1q = np.ascontiguousarray(
        w1.transpose(1, 2, 3, 0)            # [ci, kh, kw, co]
        .reshape(2, 128, 9, 256)            # [slot, ki, off, co]
        .transpose(1, 2, 0, 3)              # [ki, off, slot, co]
        .reshape(128, 9 * 2 * 256)
        .astype(f8)
    )

    def pack16(w):
        # [co, ci, kh, kw] -> [ci_t, 128, off*256 + co]
        return w.transpose(1, 2, 3, 0).reshape(2, 128, 9 * 256).astype(np.float16)

    w2c = np.ascontiguousarray(
        np.concatenate([pack16(2.0 * w2), pack16(w2)], axis=2)
    )
    return np.ascontiguousarray(x8), w1q, w2c


def kernel(x, w1, w2):
    from concourse.bass_utils import run_bass_kernel_spmd

    nc = _get_program()
    x8, w1q, w2c = _prep_inputs(x, w1, w2)
    in_maps = [
        {"x8": np.ascontiguousarray(x8[c * IMGS:(c + 1) * IMGS]),
         "w1q": w1q, "w2c": w2c}
        for c in range(N_CORES)
    ]
    res = run_bass_kernel_spmd(nc, in_maps, core_ids=list(range(N_CORES)))
    outs = [res.results[c]["out"] for c in range(N_CORES)]
    return np.concatenate(outs, axis=0).astype(np.float32)
